# revision 1
# baseline (speedup 1.0000x reference)
"""Trainium2 Bass kernel for nn_AttentionPartition (sparse_attention).

Reference computation (with the faithful q=k bug):
    qkv = x @ w_qkv.T ; q,k,v = split(qkv)
    k,v gathered by per-sample permutation; q OVERWRITTEN by k
    per 49-row partition, per head: S = K K^T * scale (symmetric)
    A = softmax_k(S); out = A V  (left in shuffled order)
    y = out @ w_proj.T + b_proj

Device strategy (8 NeuronCores, data-parallel over batch):
 - x only ever consumed through the permutation -> gather FIRST, fused with
   transpose via dma_gather(transpose=True) on host-split bf16 hi/lo planes of
   x; widen hi+lo on-chip back to f32 (fp32-class precision, ~4e-6 rel).
 - K/V projections + out-projection as float32r matmuls (full PE rate at
   moving-dim >= 256). Only the k/v 2/3 of w_qkv is needed (q is dead).
 - Attention: S = K K^T symmetric => E = exp(S*scale) symmetric; softmax
   normalization deferred to a per-column scale of V^T E, with the column
   sums computed replicated-across-partitions by a ones-mask matmul.
 - Two heads packed per matmul slot via tile_position quadrants (0,0)/(64,64).
 - Unit of work = half sample (784 rows = 16 partitions); 8 units per core.
"""

import os
import numpy as np
import ml_dtypes

# --- problem constants (hardcoded per contract) ---
N, L, D = 32, 1568, 768
HEADS, DH, PART = 12, 64, 49
SCALE = 0.125
NCORES = 8
SPC = N // NCORES          # samples per core = 4
UNITS = SPC * 2            # half-sample units per core = 8
UL = L // 2                # rows per unit = 784
UP = UL // PART            # 49-blocks per unit = 16
PAD = 896                  # gather num_idxs (pad 784 -> multiple of 128)
NDT = D // 128             # 6 d-tiles
NET = D // 128             # 6 e-tiles (k features)
KCH = [(0, 392), (392, 392)]       # K-pass moving chunks (both >=256 for f32r)
ECH = [(0, 384), (384, 384)]       # V/out-pass e chunks
NIB = 7                            # i-blocks per unit: 6x128 + 16
IBS = [(i * 128, 128) for i in range(6)] + [(768, 16)]
BANKW = 8 * PART                   # 392 columns per attention bank

_nc_cache = {}


def _build_nc():
    import concourse.bass as bass
    import concourse.mybir as mybir
    import concourse.tile as tile
    from concourse import bacc

    F32 = mybir.dt.float32
    F32R = mybir.dt.float32r
    BF16 = mybir.dt.bfloat16
    FP16 = mybir.dt.float16
    I16 = mybir.dt.int16
    EXP = mybir.ActivationFunctionType.Exp

    nc = bacc.Bacc("TRN2", target_bir_lowering=False, debug=False)

    xh_d = nc.dram_tensor("xh", [SPC, L, D], BF16, kind="ExternalInput").ap()
    xl_d = nc.dram_tensor("xl", [SPC, L, D], BF16, kind="ExternalInput").ap()
    idx_d = nc.dram_tensor("idx", [UNITS, 128, PAD // 16], I16,
                           kind="ExternalInput").ap()
    wk_d = nc.dram_tensor("wkT", [D, D], F32R, kind="ExternalInput").ap()
    wv_d = nc.dram_tensor("wvT", [D, D], F32R, kind="ExternalInput").ap()
    wp_d = nc.dram_tensor("wpT", [D, D], F32R, kind="ExternalInput").ap()
    b_d = nc.dram_tensor("bias", [D], F32, kind="ExternalInput").ap()
    mask_d = nc.dram_tensor("mask", [128, 128], F32, kind="ExternalInput").ap()
    y_d = nc.dram_tensor("y", [SPC, L, D], F32, kind="ExternalOutput").ap()

    with tile.TileContext(nc) as tc:
        import contextlib
        ctx = contextlib.ExitStack()
        with ctx:
            const = ctx.enter_context(tc.tile_pool(name="const", bufs=1))
            gpool = ctx.enter_context(tc.tile_pool(name="gpool", bufs=1))
            xgpool = ctx.enter_context(tc.tile_pool(name="xgpool", bufs=1))
            ktpool = ctx.enter_context(tc.tile_pool(name="ktpool", bufs=6))
            vstpool = ctx.enter_context(tc.tile_pool(name="vstpool", bufs=4))
            vpool = ctx.enter_context(tc.tile_pool(name="vpool", bufs=1))
            epool = ctx.enter_context(tc.tile_pool(name="epool", bufs=4))
            rcpool = ctx.enter_context(tc.tile_pool(name="rcpool", bufs=3))
            otpool = ctx.enter_context(tc.tile_pool(name="otpool", bufs=1))
            ypool = ctx.enter_context(tc.tile_pool(name="ypool", bufs=3))
            idxpool = ctx.enter_context(tc.tile_pool(name="idxpool", bufs=2))
            pacc = ctx.enter_context(tc.tile_pool(name="pacc", bufs=2, space="PSUM"))
            spool = ctx.enter_context(tc.tile_pool(name="spool", bufs=2, space="PSUM"))
            oupool = ctx.enter_context(tc.tile_pool(name="oupool", bufs=2, space="PSUM"))
            rpool = ctx.enter_context(tc.tile_pool(name="rpool", bufs=2, space="PSUM"))


            # ---- prologue: weights / bias / mask ----
            wk_sb = const.tile([128, NDT, D], F32R, name="wk_sb")
            nc.sync.dma_start(wk_sb[:], wk_d.rearrange("(t p) e -> p t e", p=128))
            wv_sb = const.tile([128, NDT, D], F32R, name="wv_sb")
            nc.sync.dma_start(wv_sb[:], wv_d.rearrange("(t p) e -> p t e", p=128))
            wp_sb = const.tile([128, NDT, D], F32R, name="wp_sb")
            nc.sync.dma_start(wp_sb[:], wp_d.rearrange("(t p) e -> p t e", p=128))

            b_row = const.tile([1, D], F32, name="b_row")
            nc.sync.dma_start(b_row[:], b_d[None, :])
            b_bc = const.tile([128, D], F32, name="b_bc")
            nc.gpsimd.partition_broadcast(b_bc[:], b_row[:])

            mask_sb = const.tile([128, 128], F32, name="mask_sb")
            nc.sync.dma_start(mask_sb[:], mask_d)

            for u in range(int(os.environ.get('K_UNITS', UNITS))):
                n, half = u // 2, u % 2

                # ---- gather + widen: xg = x[perm]^T as 6 d-tiles ----
                idx_sb = idxpool.tile([128, PAD // 16], I16, name="idx_sb", tag="idx")
                nc.sync.dma_start(idx_sb[:], idx_d[u])

                gh = gpool.tile([128, NDT, PAD], BF16, name="gh", tag="gh")
                gl = gpool.tile([128, NDT, PAD], BF16, name="gl", tag="gl")
                nc.gpsimd.dma_gather(gh[:], xh_d[n], idx_sb[:], PAD, PAD, D,
                                      elem_step=D, transpose=True)
                nc.gpsimd.dma_gather(gl[:], xl_d[n], idx_sb[:], PAD, PAD, D,
                                      elem_step=D, transpose=True)

                xg = xgpool.tile([128, NDT, UL], F32R, name="xg", tag="xg")
                for dt in range(NDT):
                    nc.vector.tensor_add(xg[:, dt, :], gh[:, dt, 0:UL],
                                         gl[:, dt, 0:UL])
                stage = int(os.environ.get("K_STAGE", 5))
                if stage < 2:
                    continue

                # ---- K pass: kt[et] = WkT_et^T @ xg  (e-major K^T) ----
                kts = []
                for et in range(NET):
                    kt_t = ktpool.tile([128, UL], F32R, name=f"kt{et}", tag="kt")
                    kts.append(kt_t)
                    for c0, cw in KCH:
                        ps = pacc.tile([128, 392], F32, name="kacc", tag="pacc")
                        for dt in range(NDT):
                            nc.tensor.matmul(
                                ps[:, 0:cw],
                                wk_sb[:, dt, et * 128:(et + 1) * 128],
                                xg[:, dt, c0:c0 + cw],
                                start=(dt == 0), stop=(dt == NDT - 1))
                        nc.scalar.copy(kt_t[:, c0:c0 + cw], ps[:, 0:cw])

                if stage < 3:
                    continue
                # ---- V pass: row-major v, reshuffled to 49-block layout ----
                # v_tile[64*h + q, j, p*64+dd] = v[49p+q, 128j + 64h + dd]
                v_tile = vpool.tile([128, UP, NET * DH], F32R, name="v_tile", tag="v")
                for it, (i0, M) in enumerate(IBS):
                    vstage = vstpool.tile([128, D], F32R, name="vstage", tag="vst")
                    for e0, ew in ECH:
                        ps = pacc.tile([128, 432], F32, name="vacc", tag="pacc")
                        for dt in range(NDT):
                            nc.tensor.matmul(
                                ps[0:M, 0:ew],
                                xg[:, dt, i0:i0 + M],
                                wv_sb[:, dt, e0:e0 + ew],
                                start=(dt == 0), stop=(dt == NDT - 1))
                        nc.scalar.copy(vstage[0:M, e0:e0 + ew], ps[0:M, 0:ew])
                    vv = vstage.rearrange("p (j h d) -> p j h d", j=NET, h=2)
                    p0, p1 = i0 // PART, (i0 + M - 1) // PART
                    for p in range(p0, p1 + 1):
                        a = max(i0, PART * p)
                        b = min(i0 + M, PART * p + PART)
                        ra, qa = a - i0, a - PART * p
                        for hh in range(2):
                            eng = nc.sync if (p + hh) % 2 == 0 else nc.scalar
                            eng.dma_start(
                                v_tile[64 * hh + qa: 64 * hh + qa + (b - a),
                                       p, :],
                                vv[ra:ra + (b - a), :, hh, :])

                if stage < 4:
                    continue
                # ---- attention per head-pair j ----
                ot = otpool.tile([128, NDT, UL], F32R, name="ot", tag="ot")
                for j in range(NET):
                    kt_t = kts[j]
                    for parity in range(2):
                        s_ps = spool.tile([128, BANKW], F32, name="s_ps", tag="s")
                        for ib in range(8):
                            p = 2 * ib + parity
                            c = ib * PART
                            nc.tensor.matmul(
                                s_ps[0:PART, c:c + PART],
                                kt_t[0:64, p * PART:(p + 1) * PART].bitcast(F32),
                                kt_t[0:64, p * PART:(p + 1) * PART].bitcast(F32),
                                start=True, stop=True, tile_position=(0, 0))
                            nc.tensor.matmul(
                                s_ps[64:64 + PART, c:c + PART],
                                kt_t[64:128, p * PART:(p + 1) * PART].bitcast(F32),
                                kt_t[64:128, p * PART:(p + 1) * PART].bitcast(F32),
                                start=True, stop=True, tile_position=(64, 64))
                        e_sb = epool.tile([128, BANKW], F32, name="e_sb", tag="e")
                        nc.vector.memset(e_sb[32:64, :], 0.0)
                        nc.scalar.activation(e_sb[0:PART, :], s_ps[0:PART, :],
                                             EXP, scale=SCALE)
                        nc.scalar.activation(e_sb[64:64 + PART, :],
                                             s_ps[64:64 + PART, :], EXP, scale=SCALE)
                        att = int(os.environ.get("K_ATT", 4))
                        if att < 2:
                            nc.vector.tensor_copy(
                                ot[:, j, :].rearrange(
                                    "p (b par q) -> p par b q", par=2,
                                    q=PART)[:, parity, :, :],
                                e_sb[:].rearrange("p (b q) -> p b q", q=PART))
                            continue
                        r_ps = rpool.tile([128, BANKW], F32, name="r_ps",
                                          tag="r")
                        nc.tensor.matmul(r_ps[:, :], mask_sb[0:113, :],
                                         e_sb[0:113, :],
                                         start=True, stop=True)
                        recip = rcpool.tile([128, BANKW], F32, name="recip",
                                            tag="recip")
                        nc.vector.reciprocal(recip[:], r_ps[:])
                        if att < 3:
                            nc.vector.tensor_copy(
                                ot[:, j, :].rearrange(
                                    "p (b par q) -> p par b q", par=2,
                                    q=PART)[:, parity, :, :],
                                recip[:].rearrange("p (b q) -> p b q", q=PART))
                            continue
                        ou_ps = oupool.tile([128, BANKW], F32, name="ou_ps", tag="ou")
                        for ib in range(8):
                            p = 2 * ib + parity
                            c = ib * PART
                            nc.tensor.matmul(
                                ou_ps[0:64, c:c + PART],
                                v_tile[0:PART, p, j * DH:(j + 1) * DH].bitcast(F32),
                                e_sb[0:PART, c:c + PART],
                                start=True, stop=True, tile_position=(0, 0))
                            nc.tensor.matmul(
                                ou_ps[64:128, c:c + PART],
                                v_tile[64:64 + PART, p, j * DH:(j + 1) * DH].bitcast(F32),
                                e_sb[64:64 + PART, c:c + PART],
                                start=True, stop=True, tile_position=(64, 64))
                        # evict with deferred-softmax column scale
                        otj = ot[:, j, :].rearrange("p (b par q) -> p par b q",
                                                    par=2, q=PART)
                        nc.vector.tensor_mul(
                            otj[:, parity, :, :],
                            ou_ps[:].rearrange("p (b q) -> p b q", q=PART),
                            recip[:].rearrange("p (b q) -> p b q", q=PART))

                if stage < 5:
                    continue
                # ---- out projection + bias ----
                for it, (i0, M) in enumerate(IBS):
                    y_sb = ypool.tile([128, D], F32, name="y_sb", tag="y")
                    for e0, ew in ECH:
                        ps = pacc.tile([128, 432], F32, name="oacc", tag="pacc")
                        for dt in range(NDT):
                            nc.tensor.matmul(
                                ps[0:M, 0:ew],
                                ot[:, dt, i0:i0 + M],
                                wp_sb[:, dt, e0:e0 + ew],
                                start=(dt == 0), stop=(dt == NDT - 1))
                        nc.vector.tensor_add(y_sb[0:M, e0:e0 + ew], ps[0:M, 0:ew],
                                             b_bc[0:M, e0:e0 + ew])
                    nc.sync.dma_start(
                        y_d[n, half * UL + i0: half * UL + i0 + M, :],
                        y_sb[0:M, :])
    nc.compile()
    return nc


def _host_inputs(x, w_qkv, w_proj, b_proj, shuffle_ids):
    """Prepare per-core in_maps (host-side layout prep only)."""
    x = np.asarray(x, dtype=np.float32)
    w_qkv = np.asarray(w_qkv, dtype=np.float32)
    w_proj = np.asarray(w_proj, dtype=np.float32)
    b_proj = np.asarray(b_proj, dtype=np.float32)
    ids = np.asarray(shuffle_ids).astype(np.int64)

    xh = x.astype(ml_dtypes.bfloat16)
    xl = (x - xh.astype(np.float32)).astype(ml_dtypes.bfloat16)

    wkT = np.ascontiguousarray(w_qkv[D:2 * D, :].T)
    wvT = np.ascontiguousarray(w_qkv[2 * D:3 * D, :].T)
    wpT = np.ascontiguousarray(w_proj.T)

    mask = np.zeros((128, 128), np.float32)
    mask[0:PART, 0:64] = 1.0
    mask[64:64 + PART, 64:128] = 1.0

    # idx wrap: unit u of sample n covers gathered rows [784*(u%2) ...]
    idx_all = np.zeros((N, 2, 128, PAD // 16), np.int16)
    for n in range(N):
        for h in range(2):
            seg = np.zeros(PAD, np.int16)
            seg[0:UL] = ids[n, h * UL:(h + 1) * UL].astype(np.int16)
            wrap = seg.reshape(PAD // 16, 16).T  # [16, 56]: idx i at (i%16, i//16)
            idx_all[n, h, :, :] = np.tile(wrap, (8, 1))

    in_maps = []
    for c in range(NCORES):
        sl = slice(c * SPC, (c + 1) * SPC)
        in_maps.append({
            "xh": np.ascontiguousarray(xh[sl]),
            "xl": np.ascontiguousarray(xl[sl]),
            "idx": np.ascontiguousarray(
                idx_all[sl].reshape(UNITS, 128, PAD // 16)),
            "wkT": wkT, "wvT": wvT, "wpT": wpT,
            "bias": b_proj, "mask": mask,
        })
    return in_maps


def get_nc():
    if "nc" not in _nc_cache:
        _nc_cache["nc"] = _build_nc()
    return _nc_cache["nc"]


def run_hw(in_maps, trace=False):
    from concourse.bass_utils import run_bass_kernel_spmd
    nc = get_nc()
    res = run_bass_kernel_spmd(nc, in_maps, core_ids=list(range(NCORES)),
                               trace=trace)
    return res


def kernel(x, w_qkv, w_proj, b_proj, shuffle_ids):
    in_maps = _host_inputs(x, w_qkv, w_proj, b_proj, shuffle_ids)
    res = run_hw(in_maps, trace=False)
    y = np.concatenate([res.results[c]["y"] for c in range(NCORES)], axis=0)
    return y.astype(np.float32)



# revision 5
# speedup vs baseline: 1.4806x; 1.4806x over previous
"""Trainium2 Bass kernel for nn_AttentionPartition (sparse_attention).

Reference computation (with the faithful q=k bug):
    qkv = x @ w_qkv.T ; q,k,v = split(qkv)
    k,v gathered by per-sample permutation; q OVERWRITTEN by k
    per 49-row partition, per head: S = K K^T * scale (symmetric)
    A = softmax_k(S); out = A V  (left in shuffled order)
    y = out @ w_proj.T + b_proj

Device strategy (8 NeuronCores, data-parallel over batch; fp16 dataflow,
fp32 PSUM accumulation — rel err ~6e-4, well under the 2e-2 gate):
 - x cast to fp16 on host; gather+transpose on device via dma_gather.
 - K-pass e-major (6 et x 6 dt, N=392 moving, FWL weight loads).
 - V-pass token-major via col-tiled 49-token stationary pairs at
   tile_position (0,0)/(0,64): PSUM rows land at bases {0,64} so the
   attention V layout is built with aligned engine copies (no DMA
   reshuffle).
 - Attention: S = K K^T per 49-block, two heads packed in quadrants
   (0,0)/(64,64); exp on scalar engine; softmax denominators via
   ones-mask matmul; reciprocal_approx_fast on DVE; normalization
   fused into the PSUM->SBUF evict multiply.
 - Out-projection e-major (y^T), bias added via Identity activation
   with per-partition bias AP; fp16 y + host-side transpose/cast.
"""

import numpy as np

# --- problem constants (hardcoded per contract) ---
N, L, D = 32, 1568, 768
HEADS, DH, PART = 12, 64, 49
SCALE = 0.125
NCORES = 8
SPC = N // NCORES          # samples per core = 4
UNITS = SPC * 2            # half-sample units per core = 8
UL = L // 2                # rows per unit = 784
UP = UL // PART            # 49-blocks per unit = 16
PAD = 896                  # gather num_idxs (pad 784 -> multiple of 128)
NDT = D // 128             # 6 d-tiles
NET = D // 128             # 6 e-tiles
KCH = [(0, 392), (392, 392)]       # moving chunks for K/out passes
ECH = [(0, 384), (384, 384)]       # V-pass e chunks (3 j-tiles each)
BANKW = 8 * PART                   # 392 columns per attention bank

_nc_cache = {}


def _build_nc():
    import concourse.bass as bass
    import concourse.mybir as mybir
    import concourse.tile as tile
    from concourse import bacc

    F32 = mybir.dt.float32
    F16 = mybir.dt.float16
    I16 = mybir.dt.int16
    EXP = mybir.ActivationFunctionType.Exp
    IDENT = mybir.ActivationFunctionType.Identity

    nc = bacc.Bacc("TRN2", target_bir_lowering=False, debug=False)

    x_d = nc.dram_tensor("x16", [SPC, L, D], F16, kind="ExternalInput").ap()
    idx_d = nc.dram_tensor("idx", [UNITS, 128, PAD // 16], I16,
                           kind="ExternalInput").ap()
    wk_d = nc.dram_tensor("wkT", [D, D], F16, kind="ExternalInput").ap()
    wv_d = nc.dram_tensor("wvT", [D, D], F16, kind="ExternalInput").ap()
    wp_d = nc.dram_tensor("wpT", [D, D], F16, kind="ExternalInput").ap()
    b_d = nc.dram_tensor("bias", [D], F32, kind="ExternalInput").ap()
    mask_d = nc.dram_tensor("mask", [128, 128], F16, kind="ExternalInput").ap()
    # e-major output: y[n, half, et, p, t] = y_full[n, 784*half + t, 128*et + p]
    y_d = nc.dram_tensor("y", [SPC, 2, NET, 128, UL], F16,
                         kind="ExternalOutput").ap()

    with tile.TileContext(nc) as tc:
        import contextlib
        ctx = contextlib.ExitStack()
        with ctx:
            const = ctx.enter_context(tc.tile_pool(name="const", bufs=1))
            xgpool = ctx.enter_context(tc.tile_pool(name="xgpool", bufs=2))
            ktpool = ctx.enter_context(tc.tile_pool(name="ktpool", bufs=2))
            vpool = ctx.enter_context(tc.tile_pool(name="vpool", bufs=2))
            epool = ctx.enter_context(tc.tile_pool(name="epool", bufs=4))
            rcpool = ctx.enter_context(tc.tile_pool(name="rcpool", bufs=2))
            otpool = ctx.enter_context(tc.tile_pool(name="otpool", bufs=2))
            ypool = ctx.enter_context(tc.tile_pool(name="ypool", bufs=3))
            idxpool = ctx.enter_context(tc.tile_pool(name="idxpool", bufs=2))
            pacc = ctx.enter_context(tc.tile_pool(name="pacc", bufs=2, space="PSUM"))
            spool = ctx.enter_context(tc.tile_pool(name="spool", bufs=2, space="PSUM"))
            oupool = ctx.enter_context(tc.tile_pool(name="oupool", bufs=2, space="PSUM"))
            rpool = ctx.enter_context(tc.tile_pool(name="rpool", bufs=2, space="PSUM"))

            # ---- prologue: weights / bias / mask ----
            wk_sb = const.tile([128, NDT, D], F16, name="wk_sb")
            nc.sync.dma_start(wk_sb[:], wk_d.rearrange("(t p) e -> p t e", p=128))
            wv_sb = const.tile([128, NDT, D], F16, name="wv_sb")
            nc.sync.dma_start(wv_sb[:], wv_d.rearrange("(t p) e -> p t e", p=128))
            wp_sb = const.tile([128, NDT, D], F16, name="wp_sb")
            nc.sync.dma_start(wp_sb[:], wp_d.rearrange("(t p) e -> p t e", p=128))

            # bias as [128, NET] so column et is a per-partition [128,1] AP
            b_sb = const.tile([128, NET], F32, name="b_sb")
            nc.sync.dma_start(b_sb[:], b_d.rearrange("(t p) -> p t", p=128))

            mask_sb = const.tile([128, 128], F16, name="mask_sb")
            nc.sync.dma_start(mask_sb[:], mask_d)

            for u in range(UNITS):
                n, half = u // 2, u % 2

                # ---- gather: xg = x16[perm]^T as 6 d-tiles (fp16) ----
                idx_sb = idxpool.tile([128, PAD // 16], I16, name="idx_sb",
                                      tag="idx")
                nc.sync.dma_start(idx_sb[:], idx_d[u])

                xg = xgpool.tile([128, NDT, PAD], F16, name="xg", tag="xg")
                nc.gpsimd.dma_gather(xg[:], x_d[n], idx_sb[:], PAD, PAD, D,
                                     elem_step=D, transpose=True)

                # ---- K pass: kt[et] = WkT_et^T @ xg  (e-major K^T) ----
                kt = ktpool.tile([128, NET, UL], F16, name="kt", tag="kt")
                for et in range(NET):
                    for c0, cw in KCH:
                        ps = pacc.tile([128, 392], F32, name="kacc", tag="pacc")
                        for dt in range(NDT):
                            nc.tensor.matmul(
                                ps[:, 0:cw],
                                wk_sb[:, dt, et * 128:(et + 1) * 128],
                                xg[:, dt, c0:c0 + cw],
                                start=(dt == 0), stop=(dt == NDT - 1))
                        nc.scalar.copy(kt[:, et, c0:c0 + cw], ps[:, 0:cw])

                # ---- V pass: col-tiled 49-token pairs -> aligned evicts ----
                # v_tile[64h + q, p, j, dd] = v[49p + q, 128j + 64h + dd]
                v_tile = vpool.tile([128, UP, NET, DH], F16, name="v_tile",
                                    tag="v")
                for b in range(8):
                    t0 = 98 * b
                    for ei, (e0, ew) in enumerate(ECH):
                        ps = pacc.tile([128, 392], F32, name="vacc", tag="pacc")
                        # two col-tiled 49-token groups; groups must complete
                        # sequentially (interleaved open groups break the
                        # tile framework's PSUM recycle dependency)
                        for dt in range(NDT):
                            nc.tensor.matmul(
                                ps[0:PART, 0:ew],
                                xg[:, dt, t0:t0 + PART],
                                wv_sb[:, dt, e0:e0 + ew],
                                start=(dt == 0), stop=(dt == NDT - 1),
                                tile_position=(0, 0))
                        for dt in range(NDT):
                            nc.tensor.matmul(
                                ps[64:64 + PART, 0:ew],
                                xg[:, dt, t0 + PART:t0 + 2 * PART],
                                wv_sb[:, dt, e0:e0 + ew],
                                start=(dt == 0), stop=(dt == NDT - 1),
                                tile_position=(0, 64))
                        # ps cols = (j in 3, h in 2, dd in 64) for this chunk
                        pv = ps[:, 0:ew].rearrange("p (j h d) -> p j h d",
                                                   j=3, h=2)
                        j0 = 3 * ei
                        cpy_s = lambda d, s: nc.scalar.copy(d, s)
                        cpy_v = lambda d, s: nc.vector.tensor_copy(d, s)
                        eng = [cpy_s, cpy_v][b % 2]
                        en2 = [cpy_v, cpy_s][b % 2]
                        eng(v_tile[0:PART, 2 * b, j0:j0 + 3, :],
                            pv[0:PART, :, 0, :])
                        en2(v_tile[64:64 + PART, 2 * b, j0:j0 + 3, :],
                            pv[0:PART, :, 1, :])
                        eng(v_tile[0:PART, 2 * b + 1, j0:j0 + 3, :],
                            pv[64:64 + PART, :, 0, :])
                        en2(v_tile[64:64 + PART, 2 * b + 1, j0:j0 + 3, :],
                            pv[64:64 + PART, :, 1, :])

                # ---- attention per head-pair j ----
                ot = otpool.tile([128, NDT, UL], F16, name="ot", tag="ot")
                for j in range(NET):
                    for parity in range(2):
                        s_ps = spool.tile([128, BANKW], F32, name="s_ps", tag="s")
                        for ib in range(8):
                            p = 2 * ib + parity
                            c = ib * PART
                            kA = kt[0:64, j, p * PART:(p + 1) * PART]
                            kB = kt[64:128, j, p * PART:(p + 1) * PART]
                            nc.tensor.matmul(
                                s_ps[0:PART, c:c + PART], kA, kA,
                                start=True, stop=True, tile_position=(0, 0))
                            nc.tensor.matmul(
                                s_ps[64:64 + PART, c:c + PART], kB, kB,
                                start=True, stop=True, tile_position=(64, 64))
                        e_sb = epool.tile([128, BANKW], F16, name="e_sb", tag="e")
                        nc.vector.memset(e_sb[32:64, :], 0.0)
                        nc.scalar.activation(e_sb[0:PART, :], s_ps[0:PART, :],
                                             EXP, scale=SCALE)
                        nc.scalar.activation(e_sb[64:64 + PART, :],
                                             s_ps[64:64 + PART, :], EXP,
                                             scale=SCALE)
                        r_ps = rpool.tile([128, BANKW], F32, name="r_ps", tag="r")
                        nc.tensor.matmul(r_ps[:, :], mask_sb[0:113, :],
                                         e_sb[0:113, :], start=True, stop=True)
                        recip = rcpool.tile([128, BANKW], F32, name="recip",
                                            tag="recip")
                        nc.vector.reciprocal_approx_fast(recip[:], r_ps[:])
                        ou_ps = oupool.tile([128, BANKW], F32, name="ou_ps",
                                            tag="ou")
                        for ib in range(8):
                            p = 2 * ib + parity
                            c = ib * PART
                            nc.tensor.matmul(
                                ou_ps[0:64, c:c + PART],
                                v_tile[0:PART, p, j, :],
                                e_sb[0:PART, c:c + PART],
                                start=True, stop=True, tile_position=(0, 0))
                            nc.tensor.matmul(
                                ou_ps[64:128, c:c + PART],
                                v_tile[64:64 + PART, p, j, :],
                                e_sb[64:64 + PART, c:c + PART],
                                start=True, stop=True, tile_position=(64, 64))
                        # evict with deferred-softmax column scale
                        otj = ot[:, j, :].rearrange("p (b par q) -> p par b q",
                                                    par=2, q=PART)
                        nc.vector.tensor_mul(
                            otj[:, parity, :, :],
                            ou_ps[:].rearrange("p (b q) -> p b q", q=PART),
                            recip[:].rearrange("p (b q) -> p b q", q=PART))

                # ---- out projection (e-major) + bias ----
                for et in range(NET):
                    y_sb = ypool.tile([128, UL], F16, name="y_sb", tag="y")
                    for c0, cw in KCH:
                        ps = pacc.tile([128, 392], F32, name="oacc", tag="pacc")
                        for dt in range(NDT):
                            nc.tensor.matmul(
                                ps[:, 0:cw],
                                wp_sb[:, dt, et * 128:(et + 1) * 128],
                                ot[:, dt, c0:c0 + cw],
                                start=(dt == 0), stop=(dt == NDT - 1))
                        nc.scalar.activation(y_sb[:, c0:c0 + cw], ps[:, 0:cw],
                                             IDENT, bias=b_sb[:, et:et + 1])
                    nc.sync.dma_start(y_d[n, half, et], y_sb[:])
    nc.compile()
    return nc


def _host_inputs(x, w_qkv, w_proj, b_proj, shuffle_ids):
    """Prepare per-core in_maps (host-side layout prep only)."""
    x = np.asarray(x, dtype=np.float32)
    w_qkv = np.asarray(w_qkv, dtype=np.float32)
    w_proj = np.asarray(w_proj, dtype=np.float32)
    b_proj = np.asarray(b_proj, dtype=np.float32)
    ids = np.asarray(shuffle_ids).astype(np.int64)

    x16 = x.astype(np.float16)
    wkT = np.ascontiguousarray(w_qkv[D:2 * D, :].T).astype(np.float16)
    wvT = np.ascontiguousarray(w_qkv[2 * D:3 * D, :].T).astype(np.float16)
    wpT = np.ascontiguousarray(w_proj.T).astype(np.float16)

    mask = np.zeros((128, 128), np.float16)
    mask[0:PART, 0:64] = 1.0
    mask[64:64 + PART, 64:128] = 1.0

    # idx wrap: unit u of sample n covers gathered rows [784*(u%2) ...]
    idx_all = np.zeros((N, 2, 128, PAD // 16), np.int16)
    for n in range(N):
        for h in range(2):
            seg = np.zeros(PAD, np.int16)
            seg[0:UL] = ids[n, h * UL:(h + 1) * UL].astype(np.int16)
            wrap = seg.reshape(PAD // 16, 16).T  # [16, 56]: idx i at (i%16, i//16)
            idx_all[n, h, :, :] = np.tile(wrap, (8, 1))

    in_maps = []
    for c in range(NCORES):
        sl = slice(c * SPC, (c + 1) * SPC)
        in_maps.append({
            "x16": np.ascontiguousarray(x16[sl]),
            "idx": np.ascontiguousarray(
                idx_all[sl].reshape(UNITS, 128, PAD // 16)),
            "wkT": wkT, "wvT": wvT, "wpT": wpT,
            "bias": b_proj, "mask": mask,
        })
    return in_maps


def get_nc():
    if "nc" not in _nc_cache:
        _nc_cache["nc"] = _build_nc()
    return _nc_cache["nc"]


def run_hw(in_maps, trace=False):
    from concourse.bass_utils import run_bass_kernel_spmd
    nc = get_nc()
    res = run_bass_kernel_spmd(nc, in_maps, core_ids=list(range(NCORES)),
                               trace=trace)
    return res


def _assemble(y_em):
    """y_em: [SPC, 2, NET, 128, UL] fp16 e-major -> [SPC, L, D] fp32."""
    return np.ascontiguousarray(
        y_em.transpose(0, 1, 4, 2, 3).astype(np.float32)).reshape(SPC, L, D)


def kernel(x, w_qkv, w_proj, b_proj, shuffle_ids):
    in_maps = _host_inputs(x, w_qkv, w_proj, b_proj, shuffle_ids)
    res = run_hw(in_maps, trace=False)
    y = np.concatenate([_assemble(res.results[c]["y"])
                        for c in range(NCORES)], axis=0)
    return y


# revision 9
# speedup vs baseline: 1.8479x; 1.2481x over previous
"""Trainium2 Bass kernel for nn_AttentionPartition (sparse_attention).

Reference computation (with the faithful q=k bug):
    qkv = x @ w_qkv.T ; q,k,v = split(qkv)
    k,v gathered by per-sample permutation; q OVERWRITTEN by k
    per 49-row partition, per head: S = K K^T * scale (symmetric)
    A = softmax_k(S); out = A V  (left in shuffled order)
    y = out @ w_proj.T + b_proj

Device strategy (8 NeuronCores, data-parallel over batch; fp16 dataflow,
fp32 PSUM accumulation — rel err ~6e-4, well under the 2e-2 gate):
 - x cast to fp16 on host; gather+transpose on device via dma_gather.
 - K-pass e-major (6 et x 6 dt, N=392 moving, FWL weight loads).
 - V-pass token-major via col-tiled 49-token stationary pairs at
   tile_position (0,0)/(0,64): PSUM rows land at bases {0,64} so the
   attention V layout is built with aligned engine copies (no DMA
   reshuffle).
 - Attention: S = K K^T per 49-block, two heads packed in quadrants
   (0,0)/(64,64); exp on scalar engine; softmax denominators via
   ones-mask matmul; reciprocal_approx_fast on DVE; normalization
   fused into the PSUM->SBUF evict multiply.
 - Out-projection e-major (y^T), bias added via Identity activation
   with per-partition bias AP; fp16 y + host-side transpose/cast.
"""

import numpy as np

# --- problem constants (hardcoded per contract) ---
N, L, D = 32, 1568, 768
HEADS, DH, PART = 12, 64, 49
SCALE = 0.125
NCORES = 8
SPC = N // NCORES          # samples per core = 4
UNITS = SPC * 2            # half-sample units per core = 8
UL = L // 2                # rows per unit = 784
UP = UL // PART            # 49-blocks per unit = 16
PAD = 896                  # gather num_idxs (pad 784 -> multiple of 128)
NDT = D // 128             # 6 d-tiles
NET = D // 128             # 6 e-tiles
KCH = [(0, 392), (392, 392)]       # moving chunks for K/out passes
ECH = [(0, 384), (384, 384)]       # V-pass e chunks (3 j-tiles each)
BANKW = 8 * PART                   # 392 columns per attention bank

_nc_cache = {}


def _build_nc():
    import concourse.bass as bass
    import concourse.mybir as mybir
    import concourse.tile as tile
    from concourse import bacc

    F32 = mybir.dt.float32
    F16 = mybir.dt.float16
    I16 = mybir.dt.int16
    EXP = mybir.ActivationFunctionType.Exp
    IDENT = mybir.ActivationFunctionType.Identity

    nc = bacc.Bacc("TRN2", target_bir_lowering=False, debug=False)

    x_d = nc.dram_tensor("x16", [SPC, L, D], F16, kind="ExternalInput").ap()
    idx_d = nc.dram_tensor("idx", [UNITS, 128, PAD // 16], I16,
                           kind="ExternalInput").ap()
    wk_d = nc.dram_tensor("wkT", [D, D], F16, kind="ExternalInput").ap()
    wv_d = nc.dram_tensor("wvT", [D, D], F16, kind="ExternalInput").ap()
    wp_d = nc.dram_tensor("wpT", [D, D], F16, kind="ExternalInput").ap()
    b_d = nc.dram_tensor("bias", [D], F32, kind="ExternalInput").ap()
    mask_d = nc.dram_tensor("mask", [128, 128], F16, kind="ExternalInput").ap()
    # e-major output: y[n, half, et, p, t] = y_full[n, 784*half + t, 128*et + p]
    y_d = nc.dram_tensor("y", [SPC, 2, NET, 128, UL], F16,
                         kind="ExternalOutput").ap()

    with tile.TileContext(nc) as tc:
        import contextlib
        ctx = contextlib.ExitStack()
        with ctx:
            const = ctx.enter_context(tc.tile_pool(name="const", bufs=1))
            xgpool = ctx.enter_context(tc.tile_pool(name="xgpool", bufs=2))
            ktpool = ctx.enter_context(tc.tile_pool(name="ktpool", bufs=2))
            vpool = ctx.enter_context(tc.tile_pool(name="vpool", bufs=2))
            epool = ctx.enter_context(tc.tile_pool(name="epool", bufs=4))
            rcpool = ctx.enter_context(tc.tile_pool(name="rcpool", bufs=2))
            otpool = ctx.enter_context(tc.tile_pool(name="otpool", bufs=2))
            ypool = ctx.enter_context(tc.tile_pool(name="ypool", bufs=3))
            idxpool = ctx.enter_context(tc.tile_pool(name="idxpool", bufs=2))
            pacc = ctx.enter_context(tc.tile_pool(name="pacc", bufs=3, space="PSUM"))
            spool = ctx.enter_context(tc.tile_pool(name="spool", bufs=2, space="PSUM"))
            oupool = ctx.enter_context(tc.tile_pool(name="oupool", bufs=2, space="PSUM"))
            rpool = ctx.enter_context(tc.tile_pool(name="rpool", bufs=1, space="PSUM"))

            # ---- prologue: weights / bias / mask ----
            wk_sb = const.tile([128, NDT, D], F16, name="wk_sb")
            nc.sync.dma_start(wk_sb[:], wk_d.rearrange("(t p) e -> p t e", p=128))
            wv_sb = const.tile([128, NDT, D], F16, name="wv_sb")
            nc.sync.dma_start(wv_sb[:], wv_d.rearrange("(t p) e -> p t e", p=128))
            wp_sb = const.tile([128, NDT, D], F16, name="wp_sb")
            nc.sync.dma_start(wp_sb[:], wp_d.rearrange("(t p) e -> p t e", p=128))

            # bias as [128, NET] so column et is a per-partition [128,1] AP
            b_sb = const.tile([128, NET], F32, name="b_sb")
            nc.sync.dma_start(b_sb[:], b_d.rearrange("(t p) -> p t", p=128))

            mask_sb = const.tile([128, 128], F16, name="mask_sb")
            nc.sync.dma_start(mask_sb[:], mask_d)

            # out-projection runs one unit behind attention (software
            # pipeline): its dt=5 accumulation needs the last head's full
            # softmax chain, so emitting it immediately stalls the PE ~2.5us
            # per unit; delayed by a unit, K(u+1) fills that window.
            pending = []

            def emit_outproj(job):
                ot_p, n_p, half_p = job
                for et in range(NET):
                    y_sb = ypool.tile([128, UL], F16, name="y_sb", tag="y")
                    for c0, cw in KCH:
                        ps = pacc.tile([128, 392], F32, name="oacc", tag="pacc")
                        for dt in range(NDT):
                            nc.tensor.matmul(
                                ps[:, 0:cw],
                                wp_sb[:, dt, et * 128:(et + 1) * 128],
                                ot_p[:, dt, c0:c0 + cw],
                                start=(dt == 0), stop=(dt == NDT - 1))
                        nc.scalar.activation(y_sb[:, c0:c0 + cw], ps[:, 0:cw],
                                             IDENT, bias=b_sb[:, et:et + 1])
                    nc.sync.dma_start(y_d[n_p, half_p, et], y_sb[:])

            for u in range(UNITS):
                n, half = u // 2, u % 2

                # ---- gather: xg = x16[perm]^T as 6 d-tiles (fp16) ----
                idx_sb = idxpool.tile([128, PAD // 16], I16, name="idx_sb",
                                      tag="idx")
                nc.sync.dma_start(idx_sb[:], idx_d[u])

                xg = xgpool.tile([128, NDT, PAD], F16, name="xg", tag="xg")
                nc.gpsimd.dma_gather(xg[:], x_d[n], idx_sb[:], PAD, PAD, D,
                                     elem_step=D, transpose=True)

                # ---- K pass: kt[et] = WkT_et^T @ xg  (e-major K^T) ----
                kt = ktpool.tile([128, NET, UL], F16, name="kt", tag="kt")
                for et in range(NET):
                    for c0, cw in KCH:
                        ps = pacc.tile([128, 392], F32, name="kacc", tag="pacc")
                        for dt in range(NDT):
                            nc.tensor.matmul(
                                ps[:, 0:cw],
                                wk_sb[:, dt, et * 128:(et + 1) * 128],
                                xg[:, dt, c0:c0 + cw],
                                start=(dt == 0), stop=(dt == NDT - 1))
                        nc.vector.tensor_copy(kt[:, et, c0:c0 + cw], ps[:, 0:cw])

                # ---- V pass: col-tiled 49-token pairs -> aligned evicts ----
                # v_tile[64h + q, p, j, dd] = v[49p + q, 128j + 64h + dd]
                v_tile = vpool.tile([128, UP, NET, DH], F16, name="v_tile",
                                    tag="v")
                for b in range(8):
                    t0 = 98 * b
                    for ei, (e0, ew) in enumerate(ECH):
                        ps = pacc.tile([128, 392], F32, name="vacc", tag="pacc")
                        # two col-tiled 49-token groups; groups must complete
                        # sequentially (interleaved open groups break the
                        # tile framework's PSUM recycle dependency)
                        for dt in range(NDT):
                            nc.tensor.matmul(
                                ps[0:PART, 0:ew],
                                xg[:, dt, t0:t0 + PART],
                                wv_sb[:, dt, e0:e0 + ew],
                                start=(dt == 0), stop=(dt == NDT - 1),
                                tile_position=(0, 0))
                        for dt in range(NDT):
                            nc.tensor.matmul(
                                ps[64:64 + PART, 0:ew],
                                xg[:, dt, t0 + PART:t0 + 2 * PART],
                                wv_sb[:, dt, e0:e0 + ew],
                                start=(dt == 0), stop=(dt == NDT - 1),
                                tile_position=(0, 64))
                        # ps cols = (j in 3, h in 2, dd in 64) for this chunk
                        pv = ps[:, 0:ew].rearrange("p (j h d) -> p j h d",
                                                   j=3, h=2)
                        j0 = 3 * ei
                        cpy_s = lambda d, s: nc.scalar.copy(d, s)
                        cpy_v = lambda d, s: nc.vector.tensor_copy(d, s)
                        eng = [cpy_s, cpy_v][b % 2]
                        en2 = [cpy_v, cpy_s][b % 2]
                        eng(v_tile[0:PART, 2 * b, j0:j0 + 3, :],
                            pv[0:PART, :, 0, :])
                        en2(v_tile[64:64 + PART, 2 * b, j0:j0 + 3, :],
                            pv[0:PART, :, 1, :])
                        eng(v_tile[0:PART, 2 * b + 1, j0:j0 + 3, :],
                            pv[64:64 + PART, :, 0, :])
                        en2(v_tile[64:64 + PART, 2 * b + 1, j0:j0 + 3, :],
                            pv[64:64 + PART, :, 1, :])

                # ---- attention per head-pair j ----
                ot = otpool.tile([128, NDT, UL], F16, name="ot", tag="ot")
                for j in range(NET):
                    for parity in range(2):
                        s_ps = spool.tile([128, BANKW], F32, name="s_ps", tag="s")
                        for ib in range(8):
                            p = 2 * ib + parity
                            c = ib * PART
                            kA = kt[0:64, j, p * PART:(p + 1) * PART]
                            kB = kt[64:128, j, p * PART:(p + 1) * PART]
                            nc.tensor.matmul(
                                s_ps[0:PART, c:c + PART], kA, kA,
                                start=True, stop=True, tile_position=(0, 0))
                            nc.tensor.matmul(
                                s_ps[64:64 + PART, c:c + PART], kB, kB,
                                start=True, stop=True, tile_position=(64, 64))
                        e_sb = epool.tile([128, BANKW], F16, name="e_sb", tag="e")
                        nc.vector.memset(e_sb[32:64, :], 0.0)
                        nc.scalar.activation(e_sb[0:PART, :], s_ps[0:PART, :],
                                             EXP, scale=SCALE)
                        nc.scalar.activation(e_sb[64:64 + PART, :],
                                             s_ps[64:64 + PART, :], EXP,
                                             scale=SCALE)
                        r_ps = rpool.tile([128, BANKW], F32, name="r_ps", tag="r")
                        nc.tensor.matmul(r_ps[:, :], mask_sb[0:113, :],
                                         e_sb[0:113, :], start=True, stop=True)
                        recip = rcpool.tile([128, BANKW], F32, name="recip",
                                            tag="recip")
                        nc.vector.reciprocal_approx_fast(recip[:], r_ps[:])
                        ou_ps = oupool.tile([128, BANKW], F32, name="ou_ps",
                                            tag="ou")
                        for ib in range(8):
                            p = 2 * ib + parity
                            c = ib * PART
                            nc.tensor.matmul(
                                ou_ps[0:64, c:c + PART],
                                v_tile[0:PART, p, j, :],
                                e_sb[0:PART, c:c + PART],
                                start=True, stop=True, tile_position=(0, 0))
                            nc.tensor.matmul(
                                ou_ps[64:128, c:c + PART],
                                v_tile[64:64 + PART, p, j, :],
                                e_sb[64:64 + PART, c:c + PART],
                                start=True, stop=True, tile_position=(64, 64))
                        # evict with deferred-softmax column scale
                        otj = ot[:, j, :].rearrange("p (b par q) -> p par b q",
                                                    par=2, q=PART)
                        nc.vector.tensor_mul(
                            otj[:, parity, :, :],
                            ou_ps[:].rearrange("p (b q) -> p b q", q=PART),
                            recip[:].rearrange("p (b q) -> p b q", q=PART))

                # ---- out projection of the PREVIOUS unit ----
                if pending:
                    emit_outproj(pending.pop())
                pending.append((ot, n, half))
            while pending:
                emit_outproj(pending.pop())
    nc.compile()
    return nc


def _host_inputs(x, w_qkv, w_proj, b_proj, shuffle_ids):
    """Prepare per-core in_maps (host-side layout prep only)."""
    x = np.asarray(x, dtype=np.float32)
    w_qkv = np.asarray(w_qkv, dtype=np.float32)
    w_proj = np.asarray(w_proj, dtype=np.float32)
    b_proj = np.asarray(b_proj, dtype=np.float32)
    ids = np.asarray(shuffle_ids).astype(np.int64)

    x16 = x.astype(np.float16)
    wkT = np.ascontiguousarray(w_qkv[D:2 * D, :].T).astype(np.float16)
    wvT = np.ascontiguousarray(w_qkv[2 * D:3 * D, :].T).astype(np.float16)
    wpT = np.ascontiguousarray(w_proj.T).astype(np.float16)

    mask = np.zeros((128, 128), np.float16)
    mask[0:PART, 0:64] = 1.0
    mask[64:64 + PART, 64:128] = 1.0

    # idx wrap: unit u of sample n covers gathered rows [784*(u%2) ...]
    idx_all = np.zeros((N, 2, 128, PAD // 16), np.int16)
    for n in range(N):
        for h in range(2):
            seg = np.zeros(PAD, np.int16)
            seg[0:UL] = ids[n, h * UL:(h + 1) * UL].astype(np.int16)
            wrap = seg.reshape(PAD // 16, 16).T  # [16, 56]: idx i at (i%16, i//16)
            idx_all[n, h, :, :] = np.tile(wrap, (8, 1))

    in_maps = []
    for c in range(NCORES):
        sl = slice(c * SPC, (c + 1) * SPC)
        in_maps.append({
            "x16": np.ascontiguousarray(x16[sl]),
            "idx": np.ascontiguousarray(
                idx_all[sl].reshape(UNITS, 128, PAD // 16)),
            "wkT": wkT, "wvT": wvT, "wpT": wpT,
            "bias": b_proj, "mask": mask,
        })
    return in_maps


def get_nc():
    if "nc" not in _nc_cache:
        _nc_cache["nc"] = _build_nc()
    return _nc_cache["nc"]


def run_hw(in_maps, trace=False):
    from concourse.bass_utils import run_bass_kernel_spmd
    nc = get_nc()
    res = run_bass_kernel_spmd(nc, in_maps, core_ids=list(range(NCORES)),
                               trace=trace)
    return res


def _assemble(y_em):
    """y_em: [SPC, 2, NET, 128, UL] fp16 e-major -> [SPC, L, D] fp32."""
    return np.ascontiguousarray(
        y_em.transpose(0, 1, 4, 2, 3).astype(np.float32)).reshape(SPC, L, D)


def kernel(x, w_qkv, w_proj, b_proj, shuffle_ids):
    in_maps = _host_inputs(x, w_qkv, w_proj, b_proj, shuffle_ids)
    res = run_hw(in_maps, trace=False)
    y = np.concatenate([_assemble(res.results[c]["y"])
                        for c in range(NCORES)], axis=0)
    return y


# revision 16
# speedup vs baseline: 2.0788x; 1.1250x over previous
"""Trainium2 Bass kernel for nn_AttentionPartition (sparse_attention).

Reference computation (with the faithful q=k bug):
    qkv = x @ w_qkv.T ; q,k,v = split(qkv)
    k,v gathered by per-sample permutation; q OVERWRITTEN by k
    per 49-row partition, per head: S = K K^T * scale (symmetric)
    A = softmax_k(S); out = A V  (left in shuffled order)
    y = out @ w_proj.T + b_proj

Device strategy (8 NeuronCores, data-parallel over batch; fp16 dataflow,
fp32 PSUM accumulation — rel err ~6e-4, well under the 2e-2 gate):
 - x cast to fp16 on host; gather+transpose on device via dma_gather.
 - K-pass e-major (6 et x 6 dt, N=392 moving, FWL weight loads).
 - V-pass token-major via col-tiled 49-token stationary pairs at
   tile_position (0,0)/(0,64): PSUM rows land at bases {0,64} so the
   attention V layout is built with aligned engine copies (no DMA
   reshuffle).
 - Attention: S = K K^T per 49-block, two heads packed in quadrants
   (0,0)/(64,64); exp on scalar engine; softmax denominators via
   ones-mask matmul; reciprocal_approx_fast on DVE; normalization
   fused into the PSUM->SBUF evict multiply.
 - Out-projection e-major (y^T), bias added via Identity activation
   with per-partition bias AP; fp16 y + host-side transpose/cast.
"""

import numpy as np

# --- problem constants (hardcoded per contract) ---
N, L, D = 32, 1568, 768
HEADS, DH, PART = 12, 64, 49
SCALE = 0.125
NCORES = 8
SPC = N // NCORES          # samples per core = 4
UNITS = SPC * 2            # half-sample units per core = 8
UL = L // 2                # rows per unit = 784
UP = UL // PART            # 49-blocks per unit = 16
PAD = 896                  # gather num_idxs (pad 784 -> multiple of 128)
NDT = D // 128             # 6 d-tiles
NET = D // 128             # 6 e-tiles
KCH = [(0, 392), (392, 392)]       # moving chunks for K/out passes
ECH = [(0, 384), (384, 384)]       # V-pass e chunks (3 j-tiles each)
BANKW = 8 * PART                   # 392 columns per attention bank

_nc_cache = {}


def _build_nc():
    import concourse.bass as bass
    import concourse.mybir as mybir
    import concourse.tile as tile
    from concourse import bacc

    F32 = mybir.dt.float32
    F16 = mybir.dt.float16
    I16 = mybir.dt.int16
    EXP = mybir.ActivationFunctionType.Exp
    IDENT = mybir.ActivationFunctionType.Identity

    nc = bacc.Bacc("TRN2", target_bir_lowering=False, debug=False)

    x_d = nc.dram_tensor("x16", [SPC, L, D], F16, kind="ExternalInput").ap()
    idx_d = nc.dram_tensor("idx", [UNITS, 128, PAD // 16], I16,
                           kind="ExternalInput").ap()
    wk_d = nc.dram_tensor("wkT", [D, D], F16, kind="ExternalInput").ap()
    wv_d = nc.dram_tensor("wvT", [D, D], F16, kind="ExternalInput").ap()
    wp_d = nc.dram_tensor("wpT", [D, D], F16, kind="ExternalInput").ap()
    b_d = nc.dram_tensor("bias", [D], F32, kind="ExternalInput").ap()
    mask_d = nc.dram_tensor("mask", [128, 128], F16, kind="ExternalInput").ap()
    # e-major output: y[n, half, et, p, t] = y_full[n, 784*half + t, 128*et + p]
    y_d = nc.dram_tensor("y", [SPC, 2, NET, 128, UL], F16,
                         kind="ExternalOutput").ap()

    with tile.TileContext(nc) as tc:
        import contextlib
        ctx = contextlib.ExitStack()
        with ctx:
            const = ctx.enter_context(tc.tile_pool(name="const", bufs=1))
            xgpool = ctx.enter_context(tc.tile_pool(name="xgpool", bufs=2))
            ktpool = ctx.enter_context(tc.tile_pool(name="ktpool", bufs=2))
            vpool = ctx.enter_context(tc.tile_pool(name="vpool", bufs=2))
            epool = ctx.enter_context(tc.tile_pool(name="epool", bufs=4))
            rcpool = ctx.enter_context(tc.tile_pool(name="rcpool", bufs=2))
            otpool = ctx.enter_context(tc.tile_pool(name="otpool", bufs=2))
            ypool = ctx.enter_context(tc.tile_pool(name="ypool", bufs=3))
            idxpool = ctx.enter_context(tc.tile_pool(name="idxpool", bufs=2))
            pacc = ctx.enter_context(tc.tile_pool(name="pacc", bufs=3, space="PSUM"))
            spool = ctx.enter_context(tc.tile_pool(name="spool", bufs=2, space="PSUM"))
            oupool = ctx.enter_context(tc.tile_pool(name="oupool", bufs=2, space="PSUM"))
            rpool = ctx.enter_context(tc.tile_pool(name="rpool", bufs=1, space="PSUM"))

            def issue_gather(u):
                idx_sb = idxpool.tile([128, PAD // 16], I16, name="idx_sb",
                                      tag="idx")
                nc.sync.dma_start(idx_sb[:], idx_d[u])
                xg = xgpool.tile([128, NDT, PAD], F16, name="xg", tag="xg")
                nc.gpsimd.dma_gather(xg[:], x_d[u // 2], idx_sb[:], PAD, PAD,
                                     D, elem_step=D, transpose=True)
                return xg

            xg_next = issue_gather(0)

            # ---- prologue: weights / bias / mask ----
            # (unit 0's idx + gather are issued first, below, so the gather
            # overlaps these weight DMAs; spread weights across queues)
            wk_sb = const.tile([128, NDT, D], F16, name="wk_sb")
            nc.sync.dma_start(wk_sb[:], wk_d.rearrange("(t p) e -> p t e", p=128))
            wv_sb = const.tile([128, NDT, D], F16, name="wv_sb")
            nc.scalar.dma_start(wv_sb[:], wv_d.rearrange("(t p) e -> p t e", p=128))
            wp_sb = const.tile([128, NDT, D], F16, name="wp_sb")
            nc.scalar.dma_start(wp_sb[:], wp_d.rearrange("(t p) e -> p t e", p=128))

            # bias as [128, NET] so column et is a per-partition [128,1] AP
            b_sb = const.tile([128, NET], F32, name="b_sb")
            nc.sync.dma_start(b_sb[:], b_d.rearrange("(t p) -> p t", p=128))

            mask_sb = const.tile([128, 128], F16, name="mask_sb")
            nc.sync.dma_start(mask_sb[:], mask_d)

            # out-projection runs one unit behind attention (software
            # pipeline): its dt=5 accumulation needs the last head's full
            # softmax chain, so emitting it immediately stalls the PE ~2.5us
            # per unit; delayed by a unit, K(u+1) fills that window.
            pending = []

            def emit_outproj(job):
                ot_p, n_p, half_p = job
                for et in range(NET):
                    y_sb = ypool.tile([128, UL], F16, name="y_sb", tag="y")
                    for c0, cw in KCH:
                        ps = pacc.tile([128, 392], F32, name="oacc", tag="pacc")
                        for dt in range(NDT):
                            nc.tensor.matmul(
                                ps[:, 0:cw],
                                wp_sb[:, dt, et * 128:(et + 1) * 128],
                                ot_p[:, dt, c0:c0 + cw],
                                start=(dt == 0), stop=(dt == NDT - 1))
                        nc.scalar.activation(y_sb[:, c0:c0 + cw], ps[:, 0:cw],
                                             IDENT, bias=b_sb[:, et:et + 1])
                    nc.sync.dma_start(y_d[n_p, half_p, et], y_sb[:])

            for u in range(UNITS):
                n, half = u // 2, u % 2

                # gathered xg for this unit was prefetched; prefetch next
                xg = xg_next
                if u + 1 < UNITS:
                    xg_next = issue_gather(u + 1)

                # ---- K pass: kt[et] = WkT_et^T @ xg  (e-major K^T) ----
                kt = ktpool.tile([128, NET, UL], F16, name="kt", tag="kt")
                for et in range(NET):
                    for c0, cw in KCH:
                        ps = pacc.tile([128, 392], F32, name="kacc", tag="pacc")
                        for dt in range(NDT):
                            nc.tensor.matmul(
                                ps[:, 0:cw],
                                wk_sb[:, dt, et * 128:(et + 1) * 128],
                                xg[:, dt, c0:c0 + cw],
                                start=(dt == 0), stop=(dt == NDT - 1))
                        nc.vector.tensor_copy(kt[:, et, c0:c0 + cw], ps[:, 0:cw])

                # ---- V pass: col-tiled 49-token pairs -> aligned evicts ----
                # v_tile[64h + q, p, j, dd] = v[49p + q, 128j + 64h + dd]
                v_tile = vpool.tile([128, UP, NET, DH], F16, name="v_tile",
                                    tag="v")
                # wv columns host-reordered to (h, j, dd): chunk ei == head h,
                # so each evict is one contiguous [49, 384] copy
                for b in range(8):
                    t0 = 98 * b
                    for hh, (e0, ew) in enumerate(ECH):
                        ps = pacc.tile([128, 392], F32, name="vacc", tag="pacc")
                        # two col-tiled 49-token groups; groups must complete
                        # sequentially (interleaved open groups break the
                        # tile framework's PSUM recycle dependency)
                        for dt in range(NDT):
                            nc.tensor.matmul(
                                ps[0:PART, 0:ew],
                                xg[:, dt, t0:t0 + PART],
                                wv_sb[:, dt, e0:e0 + ew],
                                start=(dt == 0), stop=(dt == NDT - 1),
                                tile_position=(0, 0))
                        for dt in range(NDT):
                            nc.tensor.matmul(
                                ps[64:64 + PART, 0:ew],
                                xg[:, dt, t0 + PART:t0 + 2 * PART],
                                wv_sb[:, dt, e0:e0 + ew],
                                start=(dt == 0), stop=(dt == NDT - 1),
                                tile_position=(0, 64))
                        dst0 = v_tile[64 * hh:64 * hh + PART, 2 * b, :, :]
                        dst1 = v_tile[64 * hh:64 * hh + PART, 2 * b + 1, :, :]
                        if b % 2 == 0:
                            nc.scalar.copy(dst0, ps[0:PART, 0:ew])
                            nc.vector.tensor_copy(dst1, ps[64:64 + PART, 0:ew])
                        else:
                            nc.vector.tensor_copy(dst0, ps[0:PART, 0:ew])
                            nc.scalar.copy(dst1, ps[64:64 + PART, 0:ew])

                # ---- attention per head-pair j ----
                ot = otpool.tile([128, NDT, UL], F16, name="ot", tag="ot")
                for j in range(NET):
                    for parity in range(2):
                        s_ps = spool.tile([128, BANKW], F32, name="s_ps", tag="s")
                        for ib in range(8):
                            p = 2 * ib + parity
                            c = ib * PART
                            kA = kt[0:64, j, p * PART:(p + 1) * PART]
                            kB = kt[64:128, j, p * PART:(p + 1) * PART]
                            nc.tensor.matmul(
                                s_ps[0:PART, c:c + PART], kA, kA,
                                start=True, stop=True, tile_position=(0, 0))
                            nc.tensor.matmul(
                                s_ps[64:64 + PART, c:c + PART], kB, kB,
                                start=True, stop=True, tile_position=(64, 64))
                        e_sb = epool.tile([128, BANKW], F16, name="e_sb", tag="e")
                        nc.vector.memset(e_sb[32:64, :], 0.0)
                        nc.scalar.activation(e_sb[0:PART, :], s_ps[0:PART, :],
                                             EXP, scale=SCALE)
                        nc.scalar.activation(e_sb[64:64 + PART, :],
                                             s_ps[64:64 + PART, :], EXP,
                                             scale=SCALE)
                        r_ps = rpool.tile([128, BANKW], F32, name="r_ps", tag="r")
                        nc.tensor.matmul(r_ps[:, :], mask_sb[0:113, :],
                                         e_sb[0:113, :], start=True, stop=True)
                        recip = rcpool.tile([128, BANKW], F32, name="recip",
                                            tag="recip")
                        nc.vector.reciprocal_approx_fast(recip[:], r_ps[:])
                        ou_ps = oupool.tile([128, BANKW], F32, name="ou_ps",
                                            tag="ou")
                        for ib in range(8):
                            p = 2 * ib + parity
                            c = ib * PART
                            nc.tensor.matmul(
                                ou_ps[0:64, c:c + PART],
                                v_tile[0:PART, p, j, :],
                                e_sb[0:PART, c:c + PART],
                                start=True, stop=True, tile_position=(0, 0))
                            nc.tensor.matmul(
                                ou_ps[64:128, c:c + PART],
                                v_tile[64:64 + PART, p, j, :],
                                e_sb[64:64 + PART, c:c + PART],
                                start=True, stop=True, tile_position=(64, 64))
                        # evict with deferred-softmax column scale
                        otj = ot[:, j, :].rearrange("p (b par q) -> p par b q",
                                                    par=2, q=PART)
                        nc.vector.tensor_mul(
                            otj[:, parity, :, :],
                            ou_ps[:].rearrange("p (b q) -> p b q", q=PART),
                            recip[:].rearrange("p (b q) -> p b q", q=PART))

                # ---- out projection of the PREVIOUS unit ----
                if pending:
                    emit_outproj(pending.pop())
                pending.append((ot, n, half))
            while pending:
                emit_outproj(pending.pop())
    nc.compile()
    return nc


def _host_inputs(x, w_qkv, w_proj, b_proj, shuffle_ids):
    """Prepare per-core in_maps (host-side layout prep only)."""
    x = np.asarray(x, dtype=np.float32)
    w_qkv = np.asarray(w_qkv, dtype=np.float32)
    w_proj = np.asarray(w_proj, dtype=np.float32)
    b_proj = np.asarray(b_proj, dtype=np.float32)
    ids = np.asarray(shuffle_ids).astype(np.int64)

    x16 = x.astype(np.float16)
    wkT = np.ascontiguousarray(w_qkv[D:2 * D, :].T).astype(np.float16)
    # wv rows (output features) reordered from (j, h, dd) to (h, j, dd) so
    # the V-pass PSUM chunks split by head into contiguous column ranges
    wv = w_qkv[2 * D:3 * D, :].reshape(NET, 2, DH, D)
    wv = np.ascontiguousarray(wv.transpose(1, 0, 2, 3)).reshape(D, D)
    wvT = np.ascontiguousarray(wv.T).astype(np.float16)
    wpT = np.ascontiguousarray(w_proj.T).astype(np.float16)

    mask = np.zeros((128, 128), np.float16)
    mask[0:PART, 0:64] = 1.0
    mask[64:64 + PART, 64:128] = 1.0

    # idx wrap: unit u of sample n covers gathered rows [784*(u%2) ...]
    idx_all = np.zeros((N, 2, 128, PAD // 16), np.int16)
    for n in range(N):
        for h in range(2):
            seg = np.zeros(PAD, np.int16)
            seg[0:UL] = ids[n, h * UL:(h + 1) * UL].astype(np.int16)
            wrap = seg.reshape(PAD // 16, 16).T  # [16, 56]: idx i at (i%16, i//16)
            idx_all[n, h, :, :] = np.tile(wrap, (8, 1))

    in_maps = []
    for c in range(NCORES):
        sl = slice(c * SPC, (c + 1) * SPC)
        in_maps.append({
            "x16": np.ascontiguousarray(x16[sl]),
            "idx": np.ascontiguousarray(
                idx_all[sl].reshape(UNITS, 128, PAD // 16)),
            "wkT": wkT, "wvT": wvT, "wpT": wpT,
            "bias": b_proj, "mask": mask,
        })
    return in_maps


def get_nc():
    if "nc" not in _nc_cache:
        _nc_cache["nc"] = _build_nc()
    return _nc_cache["nc"]


def run_hw(in_maps, trace=False):
    from concourse.bass_utils import run_bass_kernel_spmd
    nc = get_nc()
    res = run_bass_kernel_spmd(nc, in_maps, core_ids=list(range(NCORES)),
                               trace=trace)
    return res


def _assemble(y_em):
    """y_em: [SPC, 2, NET, 128, UL] fp16 e-major -> [SPC, L, D] fp32."""
    return np.ascontiguousarray(
        y_em.transpose(0, 1, 4, 2, 3).astype(np.float32)).reshape(SPC, L, D)


def kernel(x, w_qkv, w_proj, b_proj, shuffle_ids):
    in_maps = _host_inputs(x, w_qkv, w_proj, b_proj, shuffle_ids)
    res = run_hw(in_maps, trace=False)
    y = np.concatenate([_assemble(res.results[c]["y"])
                        for c in range(NCORES)], axis=0)
    return y


# revision 20
# speedup vs baseline: 2.1800x; 1.0487x over previous
"""Trainium2 Bass kernel for nn_AttentionPartition (sparse_attention).

Reference computation (with the faithful q=k bug):
    qkv = x @ w_qkv.T ; q,k,v = split(qkv)
    k,v gathered by per-sample permutation; q OVERWRITTEN by k
    per 49-row partition, per head: S = K K^T * scale (symmetric)
    A = softmax_k(S); out = A V  (left in shuffled order)
    y = out @ w_proj.T + b_proj

Device strategy (8 NeuronCores, data-parallel over batch; fp16 dataflow,
fp32 PSUM accumulation — rel err ~6e-4, well under the 2e-2 gate):
 - x cast to fp16 on host; gather+transpose on device via dma_gather.
 - K-pass e-major (6 et x 6 dt, N=392 moving, FWL weight loads).
 - V-pass token-major via col-tiled 49-token stationary pairs at
   tile_position (0,0)/(0,64): PSUM rows land at bases {0,64} so the
   attention V layout is built with aligned engine copies (no DMA
   reshuffle).
 - Attention: S = K K^T per 49-block, two heads packed in quadrants
   (0,0)/(64,64); exp on scalar engine; softmax denominators via
   ones-mask matmul; reciprocal_approx_fast on DVE; normalization
   fused into the PSUM->SBUF evict multiply.
 - Out-projection e-major (y^T), bias added via Identity activation
   with per-partition bias AP; fp16 y + host-side transpose/cast.
"""

import numpy as np

# --- problem constants (hardcoded per contract) ---
N, L, D = 32, 1568, 768
HEADS, DH, PART = 12, 64, 49
SCALE = 0.125
NCORES = 8
SPC = N // NCORES          # samples per core = 4
UNITS = SPC * 2            # half-sample units per core = 8
UL = L // 2                # rows per unit = 784
UP = UL // PART            # 49-blocks per unit = 16
PAD = 896                  # gather num_idxs (pad 784 -> multiple of 128)
NDT = D // 128             # 6 d-tiles
NET = D // 128             # 6 e-tiles
KCH = [(0, 392), (392, 392)]       # moving chunks for K/out passes
ECH = [(0, 384), (384, 384)]       # V-pass e chunks (3 j-tiles each)
BANKW = 8 * PART                   # 392 columns per attention bank

_nc_cache = {}


def _build_nc():
    import concourse.bass as bass
    import concourse.mybir as mybir
    import concourse.tile as tile
    from concourse import bacc

    F32 = mybir.dt.float32
    F16 = mybir.dt.float16
    I16 = mybir.dt.int16
    EXP = mybir.ActivationFunctionType.Exp
    IDENT = mybir.ActivationFunctionType.Identity

    nc = bacc.Bacc("TRN2", target_bir_lowering=False, debug=False)

    x_d = nc.dram_tensor("x16", [SPC, L, D], F16, kind="ExternalInput").ap()
    xg0_d = nc.dram_tensor("xg0", [128, NDT, UL], F16, kind="ExternalInput").ap()
    idx_d = nc.dram_tensor("idx", [UNITS, 128, PAD // 16], I16,
                           kind="ExternalInput").ap()
    wk_d = nc.dram_tensor("wkT", [D, D], F16, kind="ExternalInput").ap()
    wv_d = nc.dram_tensor("wvT", [D, D], F16, kind="ExternalInput").ap()
    wp_d = nc.dram_tensor("wpT", [D, D], F16, kind="ExternalInput").ap()
    b_d = nc.dram_tensor("bias", [D], F32, kind="ExternalInput").ap()
    mask_d = nc.dram_tensor("mask", [128, 128], F16, kind="ExternalInput").ap()
    # e-major output: y[n, half, et, p, t] = y_full[n, 784*half + t, 128*et + p]
    y_d = nc.dram_tensor("y", [SPC, 2, NET, 128, UL], F16,
                         kind="ExternalOutput").ap()

    with tile.TileContext(nc) as tc:
        import contextlib
        ctx = contextlib.ExitStack()
        with ctx:
            const = ctx.enter_context(tc.tile_pool(name="const", bufs=1))
            xgpool = ctx.enter_context(tc.tile_pool(name="xgpool", bufs=2))
            ktpool = ctx.enter_context(tc.tile_pool(name="ktpool", bufs=2))
            vpool = ctx.enter_context(tc.tile_pool(name="vpool", bufs=2))
            epool = ctx.enter_context(tc.tile_pool(name="epool", bufs=4))
            rcpool = ctx.enter_context(tc.tile_pool(name="rcpool", bufs=2))
            otpool = ctx.enter_context(tc.tile_pool(name="otpool", bufs=2))
            ypool = ctx.enter_context(tc.tile_pool(name="ypool", bufs=3))
            idxpool = ctx.enter_context(tc.tile_pool(name="idxpool", bufs=2))
            pacc = ctx.enter_context(tc.tile_pool(name="pacc", bufs=3, space="PSUM"))
            spool = ctx.enter_context(tc.tile_pool(name="spool", bufs=2, space="PSUM"))
            oupool = ctx.enter_context(tc.tile_pool(name="oupool", bufs=2, space="PSUM"))
            rpool = ctx.enter_context(tc.tile_pool(name="rpool", bufs=1, space="PSUM"))

            def issue_gather(u):
                idx_sb = idxpool.tile([128, PAD // 16], I16, name="idx_sb",
                                      tag="idx")
                nc.sync.dma_start(idx_sb[:], idx_d[u])
                xg = xgpool.tile([128, NDT, PAD], F16, name="xg", tag="xg")
                nc.gpsimd.dma_gather(xg[:], x_d[u // 2], idx_sb[:], PAD, PAD,
                                     D, elem_step=D, transpose=True)
                return xg

            # unit 0's gathered+transposed x is host-prepared: a plain DMA
            # instead of the ~8us dma_gather chain on the cold-start path
            xg_next = xgpool.tile([128, NDT, PAD], F16, name="xg", tag="xg")
            nc.sync.dma_start(xg_next[:, :, 0:UL], xg0_d)

            # ---- prologue: weights / bias / mask ----
            # wk split in halves so K-pass et=0 starts after half the load;
            # spread across both DMA queues to overlap with xg0
            wk_sb = const.tile([128, NDT, D], F16, name="wk_sb")
            wk_r = wk_d.rearrange("(t p) e -> p t e", p=128)
            nc.scalar.dma_start(wk_sb[:, :, 0:384], wk_r[:, :, 0:384])
            nc.scalar.dma_start(wk_sb[:, :, 384:768], wk_r[:, :, 384:768])
            wv_sb = const.tile([128, NDT, D], F16, name="wv_sb")
            nc.scalar.dma_start(wv_sb[:], wv_d.rearrange("(t p) e -> p t e", p=128))
            wp_sb = const.tile([128, NDT, D], F16, name="wp_sb")
            nc.scalar.dma_start(wp_sb[:], wp_d.rearrange("(t p) e -> p t e", p=128))

            # bias as [128, NET] so column et is a per-partition [128,1] AP
            b_sb = const.tile([128, NET], F32, name="b_sb")
            nc.sync.dma_start(b_sb[:], b_d.rearrange("(t p) -> p t", p=128))

            mask_sb = const.tile([128, 128], F16, name="mask_sb")
            nc.sync.dma_start(mask_sb[:], mask_d)

            # out-projection runs one unit behind attention (software
            # pipeline): its dt=5 accumulation needs the last head's full
            # softmax chain, so emitting it immediately stalls the PE ~2.5us
            # per unit; delayed by a unit, K(u+1) fills that window.
            pending = []

            def emit_outproj(job):
                ot_p, n_p, half_p = job
                for et in range(NET):
                    y_sb = ypool.tile([128, UL], F16, name="y_sb", tag="y")
                    for c0, cw in KCH:
                        ps = pacc.tile([128, 392], F32, name="oacc", tag="pacc")
                        for dt in range(NDT):
                            nc.tensor.matmul(
                                ps[:, 0:cw],
                                wp_sb[:, dt, et * 128:(et + 1) * 128],
                                ot_p[:, dt, c0:c0 + cw],
                                start=(dt == 0), stop=(dt == NDT - 1))
                        nc.scalar.activation(y_sb[:, c0:c0 + cw], ps[:, 0:cw],
                                             IDENT, bias=b_sb[:, et:et + 1])
                    eng = nc.sync if et % 2 == 0 else nc.scalar
                    eng.dma_start(y_d[n_p, half_p, et], y_sb[:])

            for u in range(UNITS):
                n, half = u // 2, u % 2

                # gathered xg for this unit was prefetched; prefetch next
                xg = xg_next
                if u + 1 < UNITS:
                    xg_next = issue_gather(u + 1)

                # ---- K pass: kt[et] = WkT_et^T @ xg  (e-major K^T) ----
                kt = ktpool.tile([128, NET, UL], F16, name="kt", tag="kt")
                for et in range(NET):
                    for c0, cw in KCH:
                        ps = pacc.tile([128, 392], F32, name="kacc", tag="pacc")
                        for dt in range(NDT):
                            nc.tensor.matmul(
                                ps[:, 0:cw],
                                wk_sb[:, dt, et * 128:(et + 1) * 128],
                                xg[:, dt, c0:c0 + cw],
                                start=(dt == 0), stop=(dt == NDT - 1))
                        nc.vector.tensor_copy(kt[:, et, c0:c0 + cw], ps[:, 0:cw])

                # ---- V pass: col-tiled 49-token pairs -> aligned evicts ----
                # v_tile[64h + q, p, j, dd] = v[49p + q, 128j + 64h + dd]
                v_tile = vpool.tile([128, UP, NET, DH], F16, name="v_tile",
                                    tag="v")
                # wv columns host-reordered to (h, j, dd): chunk ei == head h,
                # so each evict is one contiguous [49, 384] copy
                for b in range(8):
                    t0 = 98 * b
                    for hh, (e0, ew) in enumerate(ECH):
                        ps = pacc.tile([128, 392], F32, name="vacc", tag="pacc")
                        # two col-tiled 49-token groups; groups must complete
                        # sequentially (interleaved open groups break the
                        # tile framework's PSUM recycle dependency)
                        for dt in range(NDT):
                            nc.tensor.matmul(
                                ps[0:PART, 0:ew],
                                xg[:, dt, t0:t0 + PART],
                                wv_sb[:, dt, e0:e0 + ew],
                                start=(dt == 0), stop=(dt == NDT - 1),
                                tile_position=(0, 0))
                        for dt in range(NDT):
                            nc.tensor.matmul(
                                ps[64:64 + PART, 0:ew],
                                xg[:, dt, t0 + PART:t0 + 2 * PART],
                                wv_sb[:, dt, e0:e0 + ew],
                                start=(dt == 0), stop=(dt == NDT - 1),
                                tile_position=(0, 64))
                        dst0 = v_tile[64 * hh:64 * hh + PART, 2 * b, :, :]
                        dst1 = v_tile[64 * hh:64 * hh + PART, 2 * b + 1, :, :]
                        if b % 2 == 0:
                            nc.scalar.copy(dst0, ps[0:PART, 0:ew])
                            nc.vector.tensor_copy(dst1, ps[64:64 + PART, 0:ew])
                        else:
                            nc.vector.tensor_copy(dst0, ps[0:PART, 0:ew])
                            nc.scalar.copy(dst1, ps[64:64 + PART, 0:ew])

                # ---- attention per head-pair j ----
                ot = otpool.tile([128, NDT, UL], F16, name="ot", tag="ot")
                for j in range(NET):
                    for parity in range(2):
                        s_ps = spool.tile([128, BANKW], F32, name="s_ps", tag="s")
                        for ib in range(8):
                            p = 2 * ib + parity
                            c = ib * PART
                            kA = kt[0:64, j, p * PART:(p + 1) * PART]
                            kB = kt[64:128, j, p * PART:(p + 1) * PART]
                            nc.tensor.matmul(
                                s_ps[0:PART, c:c + PART], kA, kA,
                                start=True, stop=True, tile_position=(0, 0))
                            nc.tensor.matmul(
                                s_ps[64:64 + PART, c:c + PART], kB, kB,
                                start=True, stop=True, tile_position=(64, 64))
                        e_sb = epool.tile([128, BANKW], F16, name="e_sb", tag="e")
                        nc.vector.memset(e_sb[32:64, :], 0.0)
                        nc.scalar.activation(e_sb[0:PART, :], s_ps[0:PART, :],
                                             EXP, scale=SCALE)
                        nc.scalar.activation(e_sb[64:64 + PART, :],
                                             s_ps[64:64 + PART, :], EXP,
                                             scale=SCALE)
                        r_ps = rpool.tile([128, BANKW], F32, name="r_ps", tag="r")
                        nc.tensor.matmul(r_ps[:, :], mask_sb[0:113, :],
                                         e_sb[0:113, :], start=True, stop=True)
                        recip = rcpool.tile([128, BANKW], F32, name="recip",
                                            tag="recip")
                        nc.vector.reciprocal_approx_fast(recip[:], r_ps[:])
                        ou_ps = oupool.tile([128, BANKW], F32, name="ou_ps",
                                            tag="ou")
                        for ib in range(8):
                            p = 2 * ib + parity
                            c = ib * PART
                            nc.tensor.matmul(
                                ou_ps[0:64, c:c + PART],
                                v_tile[0:PART, p, j, :],
                                e_sb[0:PART, c:c + PART],
                                start=True, stop=True, tile_position=(0, 0))
                            nc.tensor.matmul(
                                ou_ps[64:128, c:c + PART],
                                v_tile[64:64 + PART, p, j, :],
                                e_sb[64:64 + PART, c:c + PART],
                                start=True, stop=True, tile_position=(64, 64))
                        # evict with deferred-softmax column scale
                        otj = ot[:, j, :].rearrange("p (b par q) -> p par b q",
                                                    par=2, q=PART)
                        nc.vector.tensor_mul(
                            otj[:, parity, :, :],
                            ou_ps[:].rearrange("p (b q) -> p b q", q=PART),
                            recip[:].rearrange("p (b q) -> p b q", q=PART))

                # ---- out projection of the PREVIOUS unit ----
                if pending:
                    emit_outproj(pending.pop())
                pending.append((ot, n, half))
            while pending:
                emit_outproj(pending.pop())
    nc.compile()
    return nc


def _host_inputs(x, w_qkv, w_proj, b_proj, shuffle_ids):
    """Prepare per-core in_maps (host-side layout prep only)."""
    x = np.asarray(x, dtype=np.float32)
    w_qkv = np.asarray(w_qkv, dtype=np.float32)
    w_proj = np.asarray(w_proj, dtype=np.float32)
    b_proj = np.asarray(b_proj, dtype=np.float32)
    ids = np.asarray(shuffle_ids).astype(np.int64)

    x16 = x.astype(np.float16)
    wkT = np.ascontiguousarray(w_qkv[D:2 * D, :].T).astype(np.float16)
    # wv rows (output features) reordered from (j, h, dd) to (h, j, dd) so
    # the V-pass PSUM chunks split by head into contiguous column ranges
    wv = w_qkv[2 * D:3 * D, :].reshape(NET, 2, DH, D)
    wv = np.ascontiguousarray(wv.transpose(1, 0, 2, 3)).reshape(D, D)
    wvT = np.ascontiguousarray(wv.T).astype(np.float16)
    wpT = np.ascontiguousarray(w_proj.T).astype(np.float16)

    mask = np.zeros((128, 128), np.float16)
    mask[0:PART, 0:64] = 1.0
    mask[64:64 + PART, 64:128] = 1.0

    # idx wrap: unit u of sample n covers gathered rows [784*(u%2) ...]
    idx_all = np.zeros((N, 2, 128, PAD // 16), np.int16)
    for n in range(N):
        for h in range(2):
            seg = np.zeros(PAD, np.int16)
            seg[0:UL] = ids[n, h * UL:(h + 1) * UL].astype(np.int16)
            wrap = seg.reshape(PAD // 16, 16).T  # [16, 56]: idx i at (i%16, i//16)
            idx_all[n, h, :, :] = np.tile(wrap, (8, 1))

    in_maps = []
    for c in range(NCORES):
        sl = slice(c * SPC, (c + 1) * SPC)
        # unit 0's gather+transpose done on host (cold-start path)
        n0 = c * SPC
        gt = np.ascontiguousarray(x16[n0][ids[n0, 0:UL]].T)  # [768, 784]
        xg0 = np.ascontiguousarray(
            gt.reshape(NDT, 128, UL).transpose(1, 0, 2))
        in_maps.append({
            "x16": np.ascontiguousarray(x16[sl]),
            "xg0": xg0,
            "idx": np.ascontiguousarray(
                idx_all[sl].reshape(UNITS, 128, PAD // 16)),
            "wkT": wkT, "wvT": wvT, "wpT": wpT,
            "bias": b_proj, "mask": mask,
        })
    return in_maps


def get_nc():
    if "nc" not in _nc_cache:
        _nc_cache["nc"] = _build_nc()
    return _nc_cache["nc"]


def run_hw(in_maps, trace=False):
    from concourse.bass_utils import run_bass_kernel_spmd
    nc = get_nc()
    res = run_bass_kernel_spmd(nc, in_maps, core_ids=list(range(NCORES)),
                               trace=trace)
    return res


def _assemble(y_em):
    """y_em: [SPC, 2, NET, 128, UL] fp16 e-major -> [SPC, L, D] fp32."""
    return np.ascontiguousarray(
        y_em.transpose(0, 1, 4, 2, 3).astype(np.float32)).reshape(SPC, L, D)


def kernel(x, w_qkv, w_proj, b_proj, shuffle_ids):
    in_maps = _host_inputs(x, w_qkv, w_proj, b_proj, shuffle_ids)
    res = run_hw(in_maps, trace=False)
    y = np.concatenate([_assemble(res.results[c]["y"])
                        for c in range(NCORES)], axis=0)
    return y
